# revision 4
# baseline (speedup 1.0000x reference)
"""BKT model (MLP + per-chain 2-state HMM scan) on 8 Trainium2 NeuronCores.

Strategy
--------
Data-parallel over batch: core m handles batch rows [8m, 8m+8).

The reference scans T=1024 steps sequentially, but each of the 500 chains is
visited only ~2x per sequence (max 11).  Host-side we reorganize each core's
8*1024 timesteps by (chain, visit-index): the 4000 (batch,chain) segments are
pooled per core and sorted by visit count descending, so that in "round" r the
active segments are exactly a prefix.  The device then runs:

  Phase A (PE): MLP over the permuted rows: H^T = tanh(W1^T X^T + b1),
                O^T = W2^T H^T + b2, in float32r (TF32-ish, 1 cyc/row).
  Phase B (DVE/ACT): per-visit HMM quantities in probability space
                (sigmoid instead of log-softmax; exact reformulation).
  Phase C: V_max sequential rounds; each round is a fully vectorized
                [128 x c_r] update of all active segments (alpha recurrence +
                normalized output log-probs).  No gathers: all indexing is
                baked into the host-side permutation of the MLP input.

Outputs are scattered back to (b, t) order on the host.
"""

import numpy as np

import concourse.bass as bass
import concourse.tile as tile
import concourse.mybir as mybir
from concourse import bacc
from concourse.bass_utils import run_bass_kernel_spmd
from concourse.masks import make_identity

B, T, NF, NH, NK, NS = 64, 1024, 512, 512, 500, 2
NCORES, BPC, P = 8, 8, 128
F32 = mybir.dt.float32
F32R = mybir.dt.float32r
AF = mybir.ActivationFunctionType
OP = mybir.AluOpType


# ---------------------------------------------------------------------------
# host-side layout
# ---------------------------------------------------------------------------

def _build_layout(kc):
    kc = np.asarray(kc)
    counts = np.zeros((B, NK), dtype=np.int64)
    for b in range(B):
        np.add.at(counts[b], kc[b].astype(np.int64), 1)
    Vmax = int(counts.max())

    seg_order = []
    n_r = np.zeros((NCORES, Vmax), dtype=np.int64)
    for m in range(NCORES):
        cnt = counts[m * BPC:(m + 1) * BPC].reshape(-1)
        order = np.argsort(-cnt, kind="stable")
        seg_order.append(order)
        for r in range(Vmax):
            n_r[m, r] = int((cnt > r).sum())

    c_r = np.maximum(1, (n_r.max(axis=0) + 127) // 128).astype(np.int64)
    Qc = int(c_r.sum())
    pad = (-Qc) % 4
    c_r[-1] += pad
    Qc += pad
    off_r = np.concatenate([[0], np.cumsum(c_r)[:-1]]).astype(np.int64)
    return dict(Vmax=Vmax, c_r=c_r, off_r=off_r, Qc=Qc, Q=128 * Qc,
                seg_order=seg_order)


def _build_host_tensors(inputs, lay):
    kc = np.asarray(inputs["kc"]).astype(np.int64)
    corr = np.asarray(inputs["corr"]).astype(np.int64)
    FM = np.ascontiguousarray(np.asarray(inputs["FM"], dtype=np.float32))
    obs = np.asarray(inputs["obs_logits"], dtype=np.float32)
    trans = np.asarray(inputs["trans_logits"], dtype=np.float32)
    init = np.asarray(inputs["init_logits"], dtype=np.float32)

    Vmax, c_r, off_r, Qc, Q = (lay["Vmax"], lay["c_r"], lay["off_r"],
                               lay["Qc"], lay["Q"])
    FMf = FM.reshape(-1, NF)

    per_core = []
    for m in range(NCORES):
        seg = lay["seg_order"][m]
        seg_rank = np.empty(BPC * NK, dtype=np.int64)
        seg_rank[seg] = np.arange(BPC * NK)

        perm = np.zeros(Q, dtype=np.int64)
        valid = np.zeros(Q, dtype=bool)

        for bl in range(BPC):
            b = m * BPC + bl
            ord_t = np.argsort(kc[b], kind="stable")
            ch = kc[b][ord_t]
            visit = np.arange(T) - np.searchsorted(ch, ch)
            s = seg_rank[bl * NK + ch]
            q = (off_r[visit] + s // 128) * 128 + (s % 128)
            perm[q] = b * T + ord_t
            valid[q] = True

        rows = perm
        ch_of_q = kc.reshape(-1)[rows]
        y_of_q = corr.reshape(-1)[rows]

        def plane(vals):
            return np.ascontiguousarray(vals.reshape(Qc, 128).T)

        og = obs[ch_of_q]
        tg = trans[ch_of_q]
        og0 = np.concatenate([plane(og[:, 0, 0]), plane(og[:, 1, 0])], axis=1)
        og1 = np.concatenate([plane(og[:, 0, 1]), plane(og[:, 1, 1])], axis=1)
        tg0 = np.concatenate([plane(tg[:, 0, 0]), plane(tg[:, 0, 1])], axis=1)
        tg1 = np.concatenate([plane(tg[:, 1, 0]), plane(tg[:, 1, 1])], axis=1)
        sgn = plane((2.0 * y_of_q - 1.0).astype(np.float32))

        Sc = 32
        igf = np.zeros((128, 2 * Sc), dtype=np.float32)
        seg_chain = seg % NK
        sl = np.arange(BPC * NK)
        igf[sl % 128, sl // 128] = init[seg_chain, 0]
        igf[sl % 128, Sc + sl // 128] = init[seg_chain, 1]

        xT = np.ascontiguousarray(FMf[rows].T)

        per_core.append(dict(
            xT=xT,
            og0=np.ascontiguousarray(og0, dtype=np.float32),
            og1=np.ascontiguousarray(og1, dtype=np.float32),
            tg0=np.ascontiguousarray(tg0, dtype=np.float32),
            tg1=np.ascontiguousarray(tg1, dtype=np.float32),
            sgn=np.ascontiguousarray(sgn, dtype=np.float32),
            ig=igf,
            perm=perm, valid=valid,
        ))

    w1 = np.ascontiguousarray(np.asarray(inputs["W1"], np.float32))
    b1r = np.ascontiguousarray(
        np.asarray(inputs["b1"], np.float32).reshape(4, 128).T)
    w2r = np.ascontiguousarray(
        np.asarray(inputs["W2"], np.float32).reshape(4, 128, 2)
        .transpose(1, 0, 2).reshape(128, 8))
    b2 = np.ascontiguousarray(np.asarray(inputs["b2"], np.float32))
    shared = dict(w1=w1, b1r=b1r, w2r=w2r, b2=b2)
    return per_core, shared


# ---------------------------------------------------------------------------
# bass kernel
# ---------------------------------------------------------------------------

def _r2(ap, w2):
    """[128, 2*w] -> [128, 2, w] plane split."""
    return ap.rearrange("p (s w) -> p s w", s=2)


def _kernel_body(ctx, tc, lay, dram):
    nc = tc.nc
    Vmax, c_r, off_r, Qc, Q = (lay["Vmax"], lay["c_r"], lay["off_r"],
                               lay["Qc"], lay["Q"])
    NTILE = Q // 512
    cmax = int(max(c_r))

    singles = ctx.enter_context(tc.tile_pool(name="singles", bufs=1))
    xt_pool = ctx.enter_context(tc.tile_pool(name="xt", bufs=3))
    ht_pool = ctx.enter_context(tc.tile_pool(name="ht", bufs=2))
    sm_pool = ctx.enter_context(tc.tile_pool(name="sm", bufs=3))
    rpool = ctx.enter_context(tc.tile_pool(name="rounds", bufs=2))
    psum = ctx.enter_context(tc.tile_pool(name="psum", bufs=1, space="PSUM"))
    psum2 = ctx.enter_context(tc.tile_pool(name="psum2", bufs=2, space="PSUM"))

    # --- persistent tiles ---
    w1sb = singles.tile([P, 4, 512], F32R, tag="w1sb")
    nc.sync.dma_start(out=w1sb, in_=dram["w1"].rearrange("(k p) n -> p k n", p=P))
    w2sb = singles.tile([P, 8], F32R, tag="w2sb")
    nc.sync.dma_start(out=w2sb, in_=dram["w2r"])
    b1sb = singles.tile([P, 4], F32, tag="b1sb")
    nc.sync.dma_start(out=b1sb, in_=dram["b1r"])
    b2sb = singles.tile([2, 1], F32, tag="b2sb")
    nc.sync.dma_start(out=b2sb, in_=dram["b2"])
    ident = singles.tile([P, P], F32, tag="ident")
    make_identity(nc, ident)

    og0t = singles.tile([P, 2 * Qc], F32, tag="og0t")
    og1t = singles.tile([P, 2 * Qc], F32, tag="og1t")
    tg0t = singles.tile([P, 2 * Qc], F32, tag="tg0t")
    tg1t = singles.tile([P, 2 * Qc], F32, tag="tg1t")
    sgnt = singles.tile([P, Qc], F32, tag="sgnt")
    igt = singles.tile([P, 64], F32, tag="igt")
    nc.sync.dma_start(out=og0t, in_=dram["og0"])
    nc.sync.dma_start(out=og1t, in_=dram["og1"])
    nc.sync.dma_start(out=tg0t, in_=dram["tg0"])
    nc.sync.dma_start(out=tg1t, in_=dram["tg1"])
    nc.sync.dma_start(out=sgnt, in_=dram["sgn"])
    nc.sync.dma_start(out=igt, in_=dram["ig"])

    ocat = singles.tile([P, 2 * Qc], F32, tag="ocat")
    kpl = singles.tile([P, 8 * Qc], F32, tag="kpl")
    outt = singles.tile([P, 2 * Qc], F32, tag="outt")

    xTv = dram["xT"].rearrange("(k p) q -> p k q", p=P)

    # --- phase A: MLP ---
    for n in range(NTILE):
        xt = xt_pool.tile([P, 4, 512], F32R, tag="xt")
        nc.sync.dma_start(out=xt, in_=xTv[:, :, n * 512:(n + 1) * 512])
        ht = ht_pool.tile([P, 4, 512], F32R, tag="ht")
        for m in range(4):
            ph = psum.tile([P, 512], F32, tag=f"h{m}")
            for k in range(4):
                nc.tensor.matmul(
                    ph,
                    lhsT=w1sb[:, k, m * 128:(m + 1) * 128],
                    rhs=xt[:, k, :],
                    start=(k == 0), stop=(k == 3))
            nc.scalar.activation(out=ht[:, m, :], in_=ph, func=AF.Tanh,
                                 bias=b1sb[:, m:m + 1], scale=1.0)
        po = psum2.tile([2, 512], F32, tag="po")
        for k in range(4):
            nc.tensor.matmul(po, lhsT=w2sb[:, 2 * k:2 * k + 2],
                             rhs=ht[:, k, :],
                             start=(k == 0), stop=(k == 3))
        ots = sm_pool.tile([2, 512], F32, tag="ots")
        nc.vector.tensor_scalar(out=ots, in0=po, scalar1=b2sb, scalar2=None,
                                op0=OP.add)
        st8 = sm_pool.tile([8, 128], F32, tag="st8")
        nc.sync.dma_start(out=st8, in_=ots.rearrange("s (c x) -> s c x", c=4))
        pt = psum2.tile([P, 8], F32, tag="pt")
        nc.tensor.transpose(out=pt, in_=st8, identity=ident[0:8, 0:8])
        # pt[:, s*4+c4] = O_s(position 128*(4n+c4)+p)
        nc.vector.tensor_copy(
            out=_r2(ocat, Qc)[:, :, 4 * n:4 * n + 4],
            in_=pt.rearrange("p (s c) -> p s c", s=2))

    # --- phase B: per-visit quantities, probability space ---
    o2 = sm_pool.tile([P, 2 * Qc], F32, tag="o2")
    nc.vector.tensor_scalar_mul(o2, ocat, 2.0)
    ogd = sm_pool.tile([P, 2 * Qc], F32, tag="ogd")
    nc.vector.tensor_sub(ogd, og1t, og0t)
    g = sm_pool.tile([P, 6 * Qc], F32, tag="g")
    sg = sm_pool.tile([P, 6 * Qc], F32, tag="sg")
    # d_s = ogd_s - 2*O_s  -> g[2Qc:4Qc]
    nc.vector.tensor_sub(g[:, 2 * Qc:4 * Qc], ogd, o2)
    # e_s = sgn * d_s      -> g[0:2Qc]
    nc.vector.tensor_tensor(
        out=_r2(g[:, 0:2 * Qc], Qc),
        in0=_r2(g[:, 2 * Qc:4 * Qc], Qc),
        in1=sgnt.unsqueeze(1).broadcast_to([P, 2, Qc]),
        op=OP.mult)
    # tgd_j = tg0_j - tg1_j -> g[4Qc:6Qc]
    nc.vector.tensor_sub(g[:, 4 * Qc:6 * Qc], tg0t, tg1t)
    nc.scalar.activation(out=sg, in_=g, func=AF.Sigmoid)
    # sg = [pe0,pe1 | p01,p11 | T00,T01]
    k4 = kpl.rearrange("p (h q w) -> p h q w", h=2, q=4)
    # plane order (q in [0,8)): [M00, M10, p00, p01 | M01, M11, p10, p11]
    nc.vector.tensor_scalar(out=k4[:, :, 2, :], in0=_r2(sg[:, 2 * Qc:4 * Qc], Qc),
                            scalar1=-1.0, scalar2=1.0, op0=OP.mult, op1=OP.add)
    nc.vector.tensor_copy(out=k4[:, :, 3, :], in_=_r2(sg[:, 2 * Qc:4 * Qc], Qc))
    tcm = sm_pool.tile([P, 2 * Qc], F32, tag="tcm")
    nc.vector.tensor_scalar(out=tcm, in0=sg[:, 4 * Qc:6 * Qc],
                            scalar1=-1.0, scalar2=1.0, op0=OP.mult, op1=OP.add)
    nc.vector.tensor_tensor(out=k4[:, :, 0, :], in0=_r2(sg[:, 4 * Qc:6 * Qc], Qc),
                            in1=_r2(sg[:, 0:2 * Qc], Qc), op=OP.mult)
    nc.vector.tensor_tensor(out=k4[:, :, 1, :], in0=_r2(tcm, Qc),
                            in1=_r2(sg[:, 0:2 * Qc], Qc), op=OP.mult)

    # --- init state ---
    ad = sm_pool.tile([P, 32], F32, tag="ad")
    nc.vector.tensor_sub(ad, igt[:, 32:64], igt[:, 0:32])
    vinit = singles.tile([P, 64], F32, tag="vinit")
    nc.scalar.activation(out=vinit[:, 32:64], in_=ad, func=AF.Sigmoid)
    nc.vector.tensor_scalar(out=vinit[:, 0:32], in0=vinit[:, 32:64],
                            scalar1=-1.0, scalar2=1.0, op0=OP.mult, op1=OP.add)

    # --- phase C: rounds ---
    k8 = kpl.rearrange("p (q w) -> p q w", q=8)
    out3 = _r2(outt, Qc)
    prev, pstride = vinit, 32
    for r in range(Vmax):
        c = int(c_r[r]); off = int(off_r[r])
        abig = rpool.tile([P, 8 * cmax], F32, tag="abig", name=f"abig{r}")[:, 0:8 * c]
        src = (prev[:, 0:2 * pstride].rearrange("p (j w) -> p j w", j=2)
               [:, :, 0:c].unsqueeze(2).broadcast_to([P, 2, 4, c]))
        nc.vector.tensor_copy(
            out=abig.rearrange("p (j q w) -> p j q w", j=2, q=4), in_=src)
        u = rpool.tile([P, 8 * cmax], F32, tag="u", name=f"u{r}")[:, 0:8 * c]
        nc.vector.tensor_tensor(
            out=u.rearrange("p (q w) -> p q w", q=8),
            in0=abig.rearrange("p (q w) -> p q w", q=8),
            in1=k8[:, :, off:off + c], op=OP.mult)
        s4 = rpool.tile([P, 4 * cmax], F32, tag="s4", name=f"s4_{r}")[:, 0:4 * c]
        nc.vector.tensor_add(s4, u[:, 0:4 * c], u[:, 4 * c:8 * c])
        sv = rpool.tile([P, cmax], F32, tag="sv", name=f"sv{r}")[:, 0:c]
        nc.vector.tensor_add(sv, s4[:, 2 * c:3 * c], s4[:, 3 * c:4 * c])
        rv = rpool.tile([P, cmax], F32, tag="rv", name=f"rv{r}")[:, 0:c]
        nc.vector.reciprocal(rv, sv)
        v = rpool.tile([P, 4 * cmax], F32, tag="v", name=f"v{r}")[:, 0:4 * c]
        nc.vector.tensor_tensor(
            out=v.rearrange("p (q w) -> p q w", q=4),
            in0=s4.rearrange("p (q w) -> p q w", q=4),
            in1=rv.unsqueeze(1).broadcast_to([P, 4, c]), op=OP.mult)
        nc.scalar.activation(out=out3[:, :, off:off + c],
                             in_=_r2(v[:, 2 * c:4 * c], c), func=AF.Ln)
        prev, pstride = v, c

    nc.sync.dma_start(out=dram["out"], in_=outt)


def _build_nc(lay):
    from contextlib import ExitStack
    nc = bacc.Bacc("TRN2", target_bir_lowering=False, debug=False,
                   num_devices=NCORES)
    Qc, Q = lay["Qc"], lay["Q"]
    dram = {}
    def din(name, shape, dt=F32):
        dram[name] = nc.dram_tensor(name, shape, dt, kind="ExternalInput").ap()
    din("xT", [NF, Q], F32R)
    din("w1", [NF, NH], F32R)
    din("b1r", [P, 4])
    din("w2r", [P, 8], F32R)
    din("b2", [2])
    din("og0", [P, 2 * Qc])
    din("og1", [P, 2 * Qc])
    din("tg0", [P, 2 * Qc])
    din("tg1", [P, 2 * Qc])
    din("sgn", [P, Qc])
    din("ig", [P, 64])
    dram["out"] = nc.dram_tensor("out", [P, 2 * Qc], F32,
                                 kind="ExternalOutput").ap()
    with tile.TileContext(nc) as tc:
        with ExitStack() as ctx:
            _kernel_body(ctx, tc, lay, dram)
    nc.compile()
    return nc


_NC_CACHE = {}


def _get_nc(lay):
    key = tuple(int(x) for x in lay["c_r"])
    if key not in _NC_CACHE:
        _NC_CACHE[key] = _build_nc(lay)
    return _NC_CACHE[key]


# ---------------------------------------------------------------------------
# entry point
# ---------------------------------------------------------------------------

def kernel(corr, kc, FM, W1, b1, W2, b2, trans_logits, obs_logits, init_logits,
           _want_results_only=True, _trace=False):
    inputs = dict(corr=corr, kc=kc, FM=FM, W1=W1, b1=b1, W2=W2, b2=b2,
                  trans_logits=trans_logits, obs_logits=obs_logits,
                  init_logits=init_logits)
    lay = _build_layout(kc)
    nc = _get_nc(lay)
    per_core, shared = _build_host_tensors(inputs, lay)

    in_maps = []
    for m in range(NCORES):
        c = per_core[m]
        in_maps.append(dict(
            xT=c["xT"], w1=shared["w1"], b1r=shared["b1r"], w2r=shared["w2r"],
            b2=shared["b2"], og0=c["og0"], og1=c["og1"], tg0=c["tg0"],
            tg1=c["tg1"], sgn=c["sgn"], ig=c["ig"]))

    res = run_bass_kernel_spmd(nc, in_maps, core_ids=list(range(NCORES)),
                               trace=_trace)

    Qc, Q = lay["Qc"], lay["Q"]
    out = np.zeros((B * T, 2), dtype=np.float32)
    J = np.arange(Q) // 128
    p = np.arange(Q) % 128
    for m in range(NCORES):
        OUT = res.results[m]["out"]
        g = per_core[m]["perm"]; v = per_core[m]["valid"]
        out[g[v], 0] = OUT[p[v], J[v]]
        out[g[v], 1] = OUT[p[v], Qc + J[v]]
    out = out.reshape(B, T, 2)
    if _want_results_only:
        return out
    return out, res


# revision 5
# speedup vs baseline: 26.9481x; 26.9481x over previous
"""BKT model (MLP + per-chain 2-state HMM scan) on 8 Trainium2 NeuronCores.

Strategy
--------
Data-parallel over batch: core m handles batch rows [8m, 8m+8).

The reference scans T=1024 steps sequentially, but each of the 500 chains is
visited only ~2x per sequence (max 11).  Host-side we reorganize each core's
8*1024 timesteps by (chain, visit-index): the 4000 (batch,chain) segments are
pooled per core and sorted by visit count descending, so that in "round" r the
active segments are exactly a prefix.  The device then runs:

  Phase A (PE): MLP over the permuted rows: H^T = tanh(W1^T X^T + b1),
                O^T = W2^T H^T + b2, in float32r (TF32-ish, 1 cyc/row).
  Phase B (DVE/ACT): per-visit HMM quantities in probability space
                (sigmoid instead of log-softmax; exact reformulation).
  Phase C: V_max sequential rounds; each round is a fully vectorized
                [128 x c_r] update of all active segments (alpha recurrence +
                normalized output log-probs).  No gathers: all indexing is
                baked into the host-side permutation of the MLP input.

Outputs are scattered back to (b, t) order on the host.
"""

import numpy as np

import concourse.bass as bass
import concourse.tile as tile
import concourse.mybir as mybir
from concourse import bacc
from concourse.bass_utils import run_bass_kernel_spmd
from concourse.masks import make_identity

B, T, NF, NH, NK, NS = 64, 1024, 512, 512, 500, 2
NCORES, BPC, P = 8, 8, 128
F32 = mybir.dt.float32
F32R = mybir.dt.float32r
AF = mybir.ActivationFunctionType
OP = mybir.AluOpType


# ---------------------------------------------------------------------------
# host-side layout
# ---------------------------------------------------------------------------

def _build_layout(kc):
    kc = np.asarray(kc)
    counts = np.zeros((B, NK), dtype=np.int64)
    for b in range(B):
        np.add.at(counts[b], kc[b].astype(np.int64), 1)
    Vmax = int(counts.max())

    seg_order = []
    n_r = np.zeros((NCORES, Vmax), dtype=np.int64)
    for m in range(NCORES):
        cnt = counts[m * BPC:(m + 1) * BPC].reshape(-1)
        order = np.argsort(-cnt, kind="stable")
        seg_order.append(order)
        for r in range(Vmax):
            n_r[m, r] = int((cnt > r).sum())

    c_r = np.maximum(1, (n_r.max(axis=0) + 127) // 128).astype(np.int64)
    Qc = int(c_r.sum())
    pad = (-Qc) % 4
    c_r[-1] += pad
    Qc += pad
    off_r = np.concatenate([[0], np.cumsum(c_r)[:-1]]).astype(np.int64)
    return dict(Vmax=Vmax, c_r=c_r, off_r=off_r, Qc=Qc, Q=128 * Qc,
                seg_order=seg_order)


def _build_host_tensors(inputs, lay):
    kc = np.asarray(inputs["kc"]).astype(np.int64)
    corr = np.asarray(inputs["corr"]).astype(np.int64)
    FM = np.ascontiguousarray(np.asarray(inputs["FM"], dtype=np.float32))
    obs = np.asarray(inputs["obs_logits"], dtype=np.float32)
    trans = np.asarray(inputs["trans_logits"], dtype=np.float32)
    init = np.asarray(inputs["init_logits"], dtype=np.float32)

    Vmax, c_r, off_r, Qc, Q = (lay["Vmax"], lay["c_r"], lay["off_r"],
                               lay["Qc"], lay["Q"])
    FMf = FM.reshape(-1, NF)

    per_core = []
    for m in range(NCORES):
        seg = lay["seg_order"][m]
        seg_rank = np.empty(BPC * NK, dtype=np.int64)
        seg_rank[seg] = np.arange(BPC * NK)

        perm = np.zeros(Q, dtype=np.int64)
        valid = np.zeros(Q, dtype=bool)

        for bl in range(BPC):
            b = m * BPC + bl
            ord_t = np.argsort(kc[b], kind="stable")
            ch = kc[b][ord_t]
            visit = np.arange(T) - np.searchsorted(ch, ch)
            s = seg_rank[bl * NK + ch]
            q = (off_r[visit] + s // 128) * 128 + (s % 128)
            perm[q] = b * T + ord_t
            valid[q] = True

        rows = perm
        ch_of_q = kc.reshape(-1)[rows]
        y_of_q = corr.reshape(-1)[rows]

        def plane(vals):
            return np.ascontiguousarray(vals.reshape(Qc, 128).T)

        og = obs[ch_of_q]
        tg = trans[ch_of_q]
        og0 = np.concatenate([plane(og[:, 0, 0]), plane(og[:, 1, 0])], axis=1)
        og1 = np.concatenate([plane(og[:, 0, 1]), plane(og[:, 1, 1])], axis=1)
        tg0 = np.concatenate([plane(tg[:, 0, 0]), plane(tg[:, 0, 1])], axis=1)
        tg1 = np.concatenate([plane(tg[:, 1, 0]), plane(tg[:, 1, 1])], axis=1)
        sgn = plane((2.0 * y_of_q - 1.0).astype(np.float32))

        Sc = 32
        igf = np.zeros((128, 2 * Sc), dtype=np.float32)
        seg_chain = seg % NK
        sl = np.arange(BPC * NK)
        igf[sl % 128, sl // 128] = init[seg_chain, 0]
        igf[sl % 128, Sc + sl // 128] = init[seg_chain, 1]

        xT = np.ascontiguousarray(FMf[rows].T)

        per_core.append(dict(
            xT=xT,
            og0=np.ascontiguousarray(og0, dtype=np.float32),
            og1=np.ascontiguousarray(og1, dtype=np.float32),
            tg0=np.ascontiguousarray(tg0, dtype=np.float32),
            tg1=np.ascontiguousarray(tg1, dtype=np.float32),
            sgn=np.ascontiguousarray(sgn, dtype=np.float32),
            ig=igf,
            perm=perm, valid=valid,
        ))

    w1 = np.ascontiguousarray(np.asarray(inputs["W1"], np.float32))
    b1r = np.ascontiguousarray(
        np.asarray(inputs["b1"], np.float32).reshape(4, 128).T)
    w2r = np.ascontiguousarray(
        np.asarray(inputs["W2"], np.float32).reshape(4, 128, 2)
        .transpose(1, 0, 2).reshape(128, 8))
    b2 = np.ascontiguousarray(np.asarray(inputs["b2"], np.float32))
    shared = dict(w1=w1, b1r=b1r, w2r=w2r, b2=b2)
    return per_core, shared


# ---------------------------------------------------------------------------
# bass kernel
# ---------------------------------------------------------------------------

def _r2(ap, w2):
    """[128, 2*w] -> [128, 2, w] plane split."""
    return ap.rearrange("p (s w) -> p s w", s=2)


def _kernel_body(ctx, tc, lay, dram, repeat=1):
    nc = tc.nc
    Vmax, c_r, off_r, Qc, Q = (lay["Vmax"], lay["c_r"], lay["off_r"],
                               lay["Qc"], lay["Q"])
    NTILE = Q // 512
    cmax = int(max(c_r))

    singles = ctx.enter_context(tc.tile_pool(name="singles", bufs=1))
    xt_pool = ctx.enter_context(tc.tile_pool(name="xt", bufs=3))
    ht_pool = ctx.enter_context(tc.tile_pool(name="ht", bufs=2))
    sm_pool = ctx.enter_context(tc.tile_pool(name="sm", bufs=3))
    rpool = ctx.enter_context(tc.tile_pool(name="rounds", bufs=2))
    psum = ctx.enter_context(tc.tile_pool(name="psum", bufs=1, space="PSUM"))
    psum2 = ctx.enter_context(tc.tile_pool(name="psum2", bufs=2, space="PSUM"))

    ident = singles.tile([P, P], F32, tag="ident")
    make_identity(nc, ident)

    for _rep in range(repeat):
        _kernel_rep(tc, lay, dram, singles, xt_pool, ht_pool, sm_pool, rpool,
                    psum, psum2, ident)


def _kernel_rep(tc, lay, dram, singles, xt_pool, ht_pool, sm_pool, rpool,
                psum, psum2, ident):
    nc = tc.nc
    Vmax, c_r, off_r, Qc, Q = (lay["Vmax"], lay["c_r"], lay["off_r"],
                               lay["Qc"], lay["Q"])
    NTILE = Q // 512
    cmax = int(max(c_r))

    # --- persistent tiles ---
    w1sb = singles.tile([P, 4, 512], F32R, tag="w1sb")
    nc.sync.dma_start(out=w1sb, in_=dram["w1"].rearrange("(k p) n -> p k n", p=P))
    w2sb = singles.tile([P, 8], F32R, tag="w2sb")
    nc.sync.dma_start(out=w2sb, in_=dram["w2r"])
    b1sb = singles.tile([P, 4], F32, tag="b1sb")
    nc.sync.dma_start(out=b1sb, in_=dram["b1r"])
    b2sb = singles.tile([2, 1], F32, tag="b2sb")
    nc.sync.dma_start(out=b2sb, in_=dram["b2"])
    og0t = singles.tile([P, 2 * Qc], F32, tag="og0t")
    og1t = singles.tile([P, 2 * Qc], F32, tag="og1t")
    tg0t = singles.tile([P, 2 * Qc], F32, tag="tg0t")
    tg1t = singles.tile([P, 2 * Qc], F32, tag="tg1t")
    sgnt = singles.tile([P, Qc], F32, tag="sgnt")
    igt = singles.tile([P, 64], F32, tag="igt")
    nc.sync.dma_start(out=og0t, in_=dram["og0"])
    nc.sync.dma_start(out=og1t, in_=dram["og1"])
    nc.sync.dma_start(out=tg0t, in_=dram["tg0"])
    nc.sync.dma_start(out=tg1t, in_=dram["tg1"])
    nc.sync.dma_start(out=sgnt, in_=dram["sgn"])
    nc.sync.dma_start(out=igt, in_=dram["ig"])

    ocat = singles.tile([P, 2 * Qc], F32, tag="ocat")
    kpl = singles.tile([P, 8 * Qc], F32, tag="kpl")
    outt = singles.tile([P, 2 * Qc], F32, tag="outt")

    xTv = dram["xT"].rearrange("(k p) q -> p k q", p=P)

    # --- phase A: MLP ---
    for n in range(NTILE):
        xt = xt_pool.tile([P, 4, 512], F32R, tag="xt")
        nc.sync.dma_start(out=xt, in_=xTv[:, :, n * 512:(n + 1) * 512])
        ht = ht_pool.tile([P, 4, 512], F32R, tag="ht")
        for m in range(4):
            ph = psum.tile([P, 512], F32, tag=f"h{m}")
            for k in range(4):
                nc.tensor.matmul(
                    ph,
                    lhsT=w1sb[:, k, m * 128:(m + 1) * 128],
                    rhs=xt[:, k, :],
                    start=(k == 0), stop=(k == 3))
            nc.scalar.activation(out=ht[:, m, :], in_=ph, func=AF.Tanh,
                                 bias=b1sb[:, m:m + 1], scale=1.0)
        po = psum2.tile([2, 512], F32, tag="po")
        for k in range(4):
            nc.tensor.matmul(po, lhsT=w2sb[:, 2 * k:2 * k + 2],
                             rhs=ht[:, k, :],
                             start=(k == 0), stop=(k == 3))
        ots = sm_pool.tile([2, 512], F32, tag="ots")
        nc.vector.tensor_scalar(out=ots, in0=po, scalar1=b2sb, scalar2=None,
                                op0=OP.add)
        st8 = sm_pool.tile([8, 128], F32, tag="st8")
        nc.sync.dma_start(out=st8, in_=ots.rearrange("s (c x) -> s c x", c=4))
        pt = psum2.tile([P, 8], F32, tag="pt")
        nc.tensor.transpose(out=pt, in_=st8, identity=ident[0:8, 0:8])
        # pt[:, s*4+c4] = O_s(position 128*(4n+c4)+p)
        nc.vector.tensor_copy(
            out=_r2(ocat, Qc)[:, :, 4 * n:4 * n + 4],
            in_=pt.rearrange("p (s c) -> p s c", s=2))

    # --- phase B: per-visit quantities, probability space ---
    o2 = sm_pool.tile([P, 2 * Qc], F32, tag="o2")
    nc.vector.tensor_scalar_mul(o2, ocat, 2.0)
    ogd = sm_pool.tile([P, 2 * Qc], F32, tag="ogd")
    nc.vector.tensor_sub(ogd, og1t, og0t)
    g = sm_pool.tile([P, 6 * Qc], F32, tag="g")
    sg = sm_pool.tile([P, 6 * Qc], F32, tag="sg")
    # d_s = ogd_s - 2*O_s  -> g[2Qc:4Qc]
    nc.vector.tensor_sub(g[:, 2 * Qc:4 * Qc], ogd, o2)
    # e_s = sgn * d_s      -> g[0:2Qc]
    nc.vector.tensor_tensor(
        out=_r2(g[:, 0:2 * Qc], Qc),
        in0=_r2(g[:, 2 * Qc:4 * Qc], Qc),
        in1=sgnt.unsqueeze(1).broadcast_to([P, 2, Qc]),
        op=OP.mult)
    # tgd_j = tg0_j - tg1_j -> g[4Qc:6Qc]
    nc.vector.tensor_sub(g[:, 4 * Qc:6 * Qc], tg0t, tg1t)
    nc.scalar.activation(out=sg, in_=g, func=AF.Sigmoid)
    # sg = [pe0,pe1 | p01,p11 | T00,T01]
    k4 = kpl.rearrange("p (h q w) -> p h q w", h=2, q=4)
    # plane order (q in [0,8)): [M00, M10, p00, p01 | M01, M11, p10, p11]
    nc.vector.tensor_scalar(out=k4[:, :, 2, :], in0=_r2(sg[:, 2 * Qc:4 * Qc], Qc),
                            scalar1=-1.0, scalar2=1.0, op0=OP.mult, op1=OP.add)
    nc.vector.tensor_copy(out=k4[:, :, 3, :], in_=_r2(sg[:, 2 * Qc:4 * Qc], Qc))
    tcm = sm_pool.tile([P, 2 * Qc], F32, tag="tcm")
    nc.vector.tensor_scalar(out=tcm, in0=sg[:, 4 * Qc:6 * Qc],
                            scalar1=-1.0, scalar2=1.0, op0=OP.mult, op1=OP.add)
    nc.vector.tensor_tensor(out=k4[:, :, 0, :], in0=_r2(sg[:, 4 * Qc:6 * Qc], Qc),
                            in1=_r2(sg[:, 0:2 * Qc], Qc), op=OP.mult)
    nc.vector.tensor_tensor(out=k4[:, :, 1, :], in0=_r2(tcm, Qc),
                            in1=_r2(sg[:, 0:2 * Qc], Qc), op=OP.mult)

    # --- init state ---
    ad = sm_pool.tile([P, 32], F32, tag="ad")
    nc.vector.tensor_sub(ad, igt[:, 32:64], igt[:, 0:32])
    vinit = singles.tile([P, 64], F32, tag="vinit")
    nc.scalar.activation(out=vinit[:, 32:64], in_=ad, func=AF.Sigmoid)
    nc.vector.tensor_scalar(out=vinit[:, 0:32], in0=vinit[:, 32:64],
                            scalar1=-1.0, scalar2=1.0, op0=OP.mult, op1=OP.add)

    # --- phase C: rounds ---
    k8 = kpl.rearrange("p (q w) -> p q w", q=8)
    out3 = _r2(outt, Qc)
    prev, pstride = vinit, 32
    for r in range(Vmax):
        c = int(c_r[r]); off = int(off_r[r])
        abig = rpool.tile([P, 8 * cmax], F32, tag="abig", name=f"abig{r}")[:, 0:8 * c]
        src = (prev[:, 0:2 * pstride].rearrange("p (j w) -> p j w", j=2)
               [:, :, 0:c].unsqueeze(2).broadcast_to([P, 2, 4, c]))
        nc.vector.tensor_copy(
            out=abig.rearrange("p (j q w) -> p j q w", j=2, q=4), in_=src)
        u = rpool.tile([P, 8 * cmax], F32, tag="u", name=f"u{r}")[:, 0:8 * c]
        nc.vector.tensor_tensor(
            out=u.rearrange("p (q w) -> p q w", q=8),
            in0=abig.rearrange("p (q w) -> p q w", q=8),
            in1=k8[:, :, off:off + c], op=OP.mult)
        s4 = rpool.tile([P, 4 * cmax], F32, tag="s4", name=f"s4_{r}")[:, 0:4 * c]
        nc.vector.tensor_add(s4, u[:, 0:4 * c], u[:, 4 * c:8 * c])
        sv = rpool.tile([P, cmax], F32, tag="sv", name=f"sv{r}")[:, 0:c]
        nc.vector.tensor_add(sv, s4[:, 2 * c:3 * c], s4[:, 3 * c:4 * c])
        rv = rpool.tile([P, cmax], F32, tag="rv", name=f"rv{r}")[:, 0:c]
        nc.vector.reciprocal(rv, sv)
        v = rpool.tile([P, 4 * cmax], F32, tag="v", name=f"v{r}")[:, 0:4 * c]
        nc.vector.tensor_tensor(
            out=v.rearrange("p (q w) -> p q w", q=4),
            in0=s4.rearrange("p (q w) -> p q w", q=4),
            in1=rv.unsqueeze(1).broadcast_to([P, 4, c]), op=OP.mult)
        nc.scalar.activation(out=out3[:, :, off:off + c],
                             in_=_r2(v[:, 2 * c:4 * c], c), func=AF.Ln)
        prev, pstride = v, c

    nc.sync.dma_start(out=dram["out"], in_=outt)


def _build_nc(lay, repeat=1):
    from contextlib import ExitStack
    nc = bacc.Bacc("TRN2", target_bir_lowering=False, debug=False,
                   num_devices=NCORES)
    Qc, Q = lay["Qc"], lay["Q"]
    dram = {}
    def din(name, shape, dt=F32):
        dram[name] = nc.dram_tensor(name, shape, dt, kind="ExternalInput").ap()
    din("xT", [NF, Q], F32R)
    din("w1", [NF, NH], F32R)
    din("b1r", [P, 4])
    din("w2r", [P, 8], F32R)
    din("b2", [2])
    din("og0", [P, 2 * Qc])
    din("og1", [P, 2 * Qc])
    din("tg0", [P, 2 * Qc])
    din("tg1", [P, 2 * Qc])
    din("sgn", [P, Qc])
    din("ig", [P, 64])
    dram["out"] = nc.dram_tensor("out", [P, 2 * Qc], F32,
                                 kind="ExternalOutput").ap()
    with tile.TileContext(nc) as tc:
        with ExitStack() as ctx:
            _kernel_body(ctx, tc, lay, dram, repeat=repeat)
    nc.compile()
    return nc


_NC_CACHE = {}


def _get_nc(lay):
    key = tuple(int(x) for x in lay["c_r"])
    if key not in _NC_CACHE:
        _NC_CACHE[key] = _build_nc(lay)
    return _NC_CACHE[key]


# ---------------------------------------------------------------------------
# entry point
# ---------------------------------------------------------------------------

def kernel(corr, kc, FM, W1, b1, W2, b2, trans_logits, obs_logits, init_logits,
           _want_results_only=True, _trace=False):
    inputs = dict(corr=corr, kc=kc, FM=FM, W1=W1, b1=b1, W2=W2, b2=b2,
                  trans_logits=trans_logits, obs_logits=obs_logits,
                  init_logits=init_logits)
    lay = _build_layout(kc)
    nc = _get_nc(lay)
    per_core, shared = _build_host_tensors(inputs, lay)

    in_maps = []
    for m in range(NCORES):
        c = per_core[m]
        in_maps.append(dict(
            xT=c["xT"], w1=shared["w1"], b1r=shared["b1r"], w2r=shared["w2r"],
            b2=shared["b2"], og0=c["og0"], og1=c["og1"], tg0=c["tg0"],
            tg1=c["tg1"], sgn=c["sgn"], ig=c["ig"]))

    res = run_bass_kernel_spmd(nc, in_maps, core_ids=list(range(NCORES)),
                               trace=_trace)

    Qc, Q = lay["Qc"], lay["Q"]
    out = np.zeros((B * T, 2), dtype=np.float32)
    J = np.arange(Q) // 128
    p = np.arange(Q) % 128
    for m in range(NCORES):
        OUT = res.results[m]["out"]
        g = per_core[m]["perm"]; v = per_core[m]["valid"]
        out[g[v], 0] = OUT[p[v], J[v]]
        out[g[v], 1] = OUT[p[v], Qc + J[v]]
    out = out.reshape(B, T, 2)
    if _want_results_only:
        return out
    return out, res


# revision 17
# speedup vs baseline: 32.4883x; 1.2056x over previous
"""BKT model (MLP + per-chain 2-state HMM scan) on 8 Trainium2 NeuronCores.

Strategy
--------
Data-parallel over batch: core m handles batch rows [8m, 8m+8).

The reference scans T=1024 steps sequentially, but each of the 500 chains is
visited only ~2x per sequence (max 11).  Host-side we reorganize each core's
8*1024 timesteps by (chain, visit-index): the 4000 (batch,chain) segments are
pooled per core and sorted by visit count descending, so that in "round" r the
active segments are exactly a prefix.  The device then runs:

  Phase A (PE): MLP over the permuted rows: H^T = tanh(W1^T X^T + b1),
                O^T = W2^T H^T + b2, in float32r (TF32-ish, 1 cyc/row).
  Phase B (DVE/ACT): per-visit HMM quantities in probability space
                (sigmoid instead of log-softmax; exact reformulation).
  Phase C: V_max sequential rounds; each round is a fully vectorized
                [128 x c_r] update of all active segments (alpha recurrence +
                normalized output log-probs).  No gathers: all indexing is
                baked into the host-side permutation of the MLP input.

Outputs are scattered back to (b, t) order on the host.
"""

import numpy as np

import concourse.bass as bass
import concourse.tile as tile
import concourse.mybir as mybir
from concourse import bacc
from concourse.bass_utils import run_bass_kernel_spmd
from concourse.masks import make_identity

B, T, NF, NH, NK, NS = 64, 1024, 512, 512, 500, 2
NCORES, BPC, P = 8, 8, 128
F32 = mybir.dt.float32
F32R = mybir.dt.float32r
AF = mybir.ActivationFunctionType
OP = mybir.AluOpType


# ---------------------------------------------------------------------------
# host-side layout
# ---------------------------------------------------------------------------

def _build_layout(kc):
    kc = np.asarray(kc)
    counts = np.zeros((B, NK), dtype=np.int64)
    for b in range(B):
        np.add.at(counts[b], kc[b].astype(np.int64), 1)
    Vmax = int(counts.max())

    seg_order = []
    n_r = np.zeros((NCORES, Vmax), dtype=np.int64)
    for m in range(NCORES):
        cnt = counts[m * BPC:(m + 1) * BPC].reshape(-1)
        order = np.argsort(-cnt, kind="stable")
        seg_order.append(order)
        for r in range(Vmax):
            n_r[m, r] = int((cnt > r).sum())

    c_r = np.maximum(1, (n_r.max(axis=0) + 127) // 128).astype(np.int64)
    Qc = int(c_r.sum())
    pad = (-Qc) % 4
    c_r[-1] += pad
    Qc += pad
    off_r = np.concatenate([[0], np.cumsum(c_r)[:-1]]).astype(np.int64)
    # chunks: unions of consecutive rounds whose end column is a multiple of 4
    # (so each 512-position matmul tile maps to exactly one chunk)
    chunks = []
    start_r = 0
    for r in range(Vmax):
        end_col = int(off_r[r] + c_r[r])
        if end_col % 4 == 0:
            col0 = int(off_r[start_r])
            chunks.append((start_r, r + 1, col0, end_col - col0))
            start_r = r + 1
    assert start_r == Vmax
    return dict(Vmax=Vmax, c_r=c_r, off_r=off_r, Qc=Qc, Q=128 * Qc,
                seg_order=seg_order, chunks=chunks)


def _build_host_tensors(inputs, lay):
    kc = np.asarray(inputs["kc"]).astype(np.int64)
    corr = np.asarray(inputs["corr"]).astype(np.int64)
    FM = np.ascontiguousarray(np.asarray(inputs["FM"], dtype=np.float32))
    obs = np.asarray(inputs["obs_logits"], dtype=np.float32)
    trans = np.asarray(inputs["trans_logits"], dtype=np.float32)
    init = np.asarray(inputs["init_logits"], dtype=np.float32)

    Vmax, c_r, off_r, Qc, Q = (lay["Vmax"], lay["c_r"], lay["off_r"],
                               lay["Qc"], lay["Q"])
    FMf = FM.reshape(-1, NF)

    per_core = []
    for m in range(NCORES):
        seg = lay["seg_order"][m]
        seg_rank = np.empty(BPC * NK, dtype=np.int64)
        seg_rank[seg] = np.arange(BPC * NK)

        perm = np.zeros(Q, dtype=np.int64)
        valid = np.zeros(Q, dtype=bool)

        for bl in range(BPC):
            b = m * BPC + bl
            ord_t = np.argsort(kc[b], kind="stable")
            ch = kc[b][ord_t]
            visit = np.arange(T) - np.searchsorted(ch, ch)
            s = seg_rank[bl * NK + ch]
            q = (off_r[visit] + s // 128) * 128 + (s % 128)
            perm[q] = b * T + ord_t
            valid[q] = True

        rows = perm
        ch_of_q = kc.reshape(-1)[rows]
        y_of_q = corr.reshape(-1)[rows]

        def plane(vals):
            return np.ascontiguousarray(vals.reshape(Qc, 128).T)

        og = obs[ch_of_q]
        tg = trans[ch_of_q]
        og0 = np.concatenate([plane(og[:, 0, 0]), plane(og[:, 1, 0])], axis=1)
        og1 = np.concatenate([plane(og[:, 0, 1]), plane(og[:, 1, 1])], axis=1)
        tg0 = np.concatenate([plane(tg[:, 0, 0]), plane(tg[:, 0, 1])], axis=1)
        tg1 = np.concatenate([plane(tg[:, 1, 0]), plane(tg[:, 1, 1])], axis=1)
        sgn = plane((2.0 * y_of_q - 1.0).astype(np.float32))

        Sc = 32
        igf = np.zeros((128, 2 * Sc), dtype=np.float32)
        seg_chain = seg % NK
        sl = np.arange(BPC * NK)
        igf[sl % 128, sl // 128] = init[seg_chain, 0]
        igf[sl % 128, Sc + sl // 128] = init[seg_chain, 1]

        xT = np.ascontiguousarray(FMf[rows].T)

        per_core.append(dict(
            xT=xT,
            og0=np.ascontiguousarray(og0, dtype=np.float32),
            og1=np.ascontiguousarray(og1, dtype=np.float32),
            tg0=np.ascontiguousarray(tg0, dtype=np.float32),
            tg1=np.ascontiguousarray(tg1, dtype=np.float32),
            sgn=np.ascontiguousarray(sgn, dtype=np.float32),
            ig=igf,
            perm=perm, valid=valid,
        ))

    w1 = np.ascontiguousarray(np.asarray(inputs["W1"], np.float32))
    b1r = np.ascontiguousarray(
        np.asarray(inputs["b1"], np.float32).reshape(4, 128).T)
    w2r = np.ascontiguousarray(
        np.asarray(inputs["W2"], np.float32).reshape(4, 128, 2)
        .transpose(1, 0, 2).reshape(128, 8))
    b2 = np.ascontiguousarray(np.asarray(inputs["b2"], np.float32))
    shared = dict(w1=w1, b1r=b1r, w2r=w2r, b2=b2)
    return per_core, shared


# ---------------------------------------------------------------------------
# bass kernel
# ---------------------------------------------------------------------------

def _r2(ap, w2):
    """[128, 2*w] -> [128, 2, w] plane split."""
    return ap.rearrange("p (s w) -> p s w", s=2)


def _kernel_body(ctx, tc, lay, dram, repeat=1):
    nc = tc.nc
    Vmax, c_r, off_r, Qc, Q = (lay["Vmax"], lay["c_r"], lay["off_r"],
                               lay["Qc"], lay["Q"])
    NTILE = Q // 512
    cmax = int(max(c_r))

    singles = ctx.enter_context(tc.tile_pool(name="singles", bufs=1))
    xt_pool = ctx.enter_context(tc.tile_pool(name="xt", bufs=4))
    ht_pool = ctx.enter_context(tc.tile_pool(name="ht", bufs=2))
    sm_pool = ctx.enter_context(tc.tile_pool(name="sm", bufs=3))
    rpool = ctx.enter_context(tc.tile_pool(name="rounds", bufs=2))
    psum = ctx.enter_context(tc.tile_pool(name="psum", bufs=1, space="PSUM"))
    psum2 = ctx.enter_context(tc.tile_pool(name="psum2", bufs=2, space="PSUM"))

    ident = singles.tile([P, P], F32, tag="ident")
    make_identity(nc, ident)

    for _rep in range(repeat):
        _kernel_rep(tc, lay, dram, singles, xt_pool, ht_pool, sm_pool, rpool,
                    psum, psum2, ident)


def _kernel_rep(tc, lay, dram, singles, xt_pool, ht_pool, sm_pool, rpool,
                psum, psum2, ident):
    nc = tc.nc
    Vmax, c_r, off_r, Qc, Q = (lay["Vmax"], lay["c_r"], lay["off_r"],
                               lay["Qc"], lay["Q"])
    NTILE = Q // 512
    cmax = int(max(c_r))
    chunks = lay["chunks"]

    # --- small weights early on the ACT HWDGE ring; xt owns the SP ring ---
    w1v = dram["w1"].rearrange("(k p) n -> p k n", p=P)
    w1sb = [singles.tile([P, 512], F32R, tag=f"w1sb{k}", name=f"w1sb{k}")
            for k in range(4)]
    for k in range(4):
        nc.scalar.dma_start(out=w1sb[k], in_=w1v[:, k, :])
    w2sb = singles.tile([P, 8], F32R, tag="w2sb")
    nc.scalar.dma_start(out=w2sb, in_=dram["w2r"])
    b1sb = singles.tile([P, 4], F32, tag="b1sb")
    nc.scalar.dma_start(out=b1sb, in_=dram["b1r"])
    b2sb = singles.tile([2, 1], F32, tag="b2sb")
    nc.scalar.dma_start(out=b2sb, in_=dram["b2"])

    og0t = singles.tile([P, 2 * Qc], F32, tag="og0t")
    og1t = singles.tile([P, 2 * Qc], F32, tag="og1t")
    tg0t = singles.tile([P, 2 * Qc], F32, tag="tg0t")
    tg1t = singles.tile([P, 2 * Qc], F32, tag="tg1t")
    sgnt = singles.tile([P, Qc], F32, tag="sgnt")
    igt = singles.tile([P, 64], F32, tag="igt")

    outt = singles.tile([P, 2 * Qc], F32, tag="outt")
    pyt = singles.tile([P, 3 * Qc], F32, tag="pyt")
    py3 = pyt.rearrange("p (s w) -> p s w", s=3)
    out3 = _r2(outt, Qc)
    xTv = dram["xT"].rearrange("(k p) q -> p k q", p=P)

    ocat_ch = [singles.tile([P, 2 * w], F32, tag=f"ocat{ci}", name=f"ocat{ci}")
               for ci, (_, _, _, w) in enumerate(chunks)]
    kpl_ch = [singles.tile([P, 8 * w], F32, tag=f"kpl{ci}", name=f"kpl{ci}")
              for ci, (_, _, _, w) in enumerate(chunks)]
    chunk_of_col = np.zeros(Qc, dtype=np.int64)
    for ci, (_, _, col0, w) in enumerate(chunks):
        chunk_of_col[col0:col0 + w] = ci

    state = dict(prev=None, pstride=32)

    def emit_plane_loads():
        nc.scalar.dma_start(out=og0t, in_=dram["og0"])
        nc.scalar.dma_start(out=og1t, in_=dram["og1"])
        nc.scalar.dma_start(out=tg0t, in_=dram["tg0"])
        nc.scalar.dma_start(out=tg1t, in_=dram["tg1"])
        nc.scalar.dma_start(out=sgnt, in_=dram["sgn"])
        nc.scalar.dma_start(out=igt, in_=dram["ig"])
        # init state: a1 = sigmoid(ig1-ig0) = 0.5 + 0.5*tanh((ig1-ig0)/2)
        ad = sm_pool.tile([P, 32], F32, tag="ad", name="ad")
        nc.vector.tensor_sub(ad, igt[:, 32:64], igt[:, 0:32])
        th = sm_pool.tile([P, 32], F32, tag="th", name="th")
        nc.scalar.activation(out=th, in_=ad, func=AF.Tanh, scale=0.5)
        vinit = singles.tile([P, 64], F32, tag="vinit")
        nc.vector.tensor_scalar(out=vinit[:, 32:64], in0=th,
                                scalar1=0.5, scalar2=0.5,
                                op0=OP.mult, op1=OP.add)
        nc.vector.tensor_scalar(out=vinit[:, 0:32], in0=th,
                                scalar1=-0.5, scalar2=0.5,
                                op0=OP.mult, op1=OP.add)
        state["prev"] = vinit

    def phase_b_and_rounds(ci):
        r0, r1, col0, w = chunks[ci]
        oc = ocat_ch[ci]
        o2c = sm_pool.tile([P, 2 * cmax], F32, tag="o2c",
                           name=f"o2c{ci}")[:, 0:2 * w]
        nc.vector.tensor_scalar_mul(o2c, oc, 2.0)
        ogdc = sm_pool.tile([P, 2 * cmax], F32, tag="ogdc",
                            name=f"ogdc{ci}")[:, 0:2 * w]
        nc.vector.tensor_tensor(out=_r2(ogdc, w),
                                in0=_r2(og1t, Qc)[:, :, col0:col0 + w],
                                in1=_r2(og0t, Qc)[:, :, col0:col0 + w],
                                op=OP.subtract)
        g = sm_pool.tile([P, 6 * cmax], F32, tag="g", name=f"g{ci}")[:, 0:6 * w]
        sg = sm_pool.tile([P, 6 * cmax], F32, tag="sg",
                          name=f"sg{ci}")[:, 0:6 * w]
        nc.vector.tensor_sub(g[:, 2 * w:4 * w], ogdc, o2c)
        nc.vector.tensor_tensor(
            out=_r2(g[:, 0:2 * w], w), in0=_r2(g[:, 2 * w:4 * w], w),
            in1=sgnt[:, col0:col0 + w].unsqueeze(1).broadcast_to([P, 2, w]),
            op=OP.mult)
        nc.vector.tensor_tensor(out=_r2(g[:, 4 * w:6 * w], w),
                                in0=_r2(tg0t, Qc)[:, :, col0:col0 + w],
                                in1=_r2(tg1t, Qc)[:, :, col0:col0 + w],
                                op=OP.subtract)
        # sigmoid(x) = 0.5 + 0.5*tanh(x/2): keep ACT on the tanh table set
        nc.scalar.activation(out=sg, in_=g, func=AF.Tanh, scale=0.5)
        nc.vector.tensor_scalar(out=sg, in0=sg, scalar1=0.5, scalar2=0.5,
                                op0=OP.mult, op1=OP.add)
        # sg = [pe0,pe1 | p01,p11 | T00,T01] (probabilities)
        kt = kpl_ch[ci]
        k4 = kt.rearrange("p (h q w) -> p h q w", h=2, q=4)
        nc.vector.tensor_scalar(out=k4[:, :, 2, :], in0=_r2(sg[:, 2 * w:4 * w], w),
                                scalar1=-1.0, scalar2=1.0,
                                op0=OP.mult, op1=OP.add)
        nc.vector.tensor_copy(out=k4[:, :, 3, :], in_=_r2(sg[:, 2 * w:4 * w], w))
        tcm = sm_pool.tile([P, 2 * cmax], F32, tag="tcm",
                           name=f"tcm{ci}")[:, 0:2 * w]
        nc.vector.tensor_scalar(out=tcm, in0=sg[:, 4 * w:6 * w],
                                scalar1=-1.0, scalar2=1.0,
                                op0=OP.mult, op1=OP.add)
        nc.vector.tensor_tensor(out=k4[:, :, 0, :], in0=_r2(sg[:, 4 * w:6 * w], w),
                                in1=_r2(sg[:, 0:2 * w], w), op=OP.mult)
        nc.vector.tensor_tensor(out=k4[:, :, 1, :], in0=_r2(tcm, w),
                                in1=_r2(sg[:, 0:2 * w], w), op=OP.mult)

        k4v = kt.rearrange("p (j q w) -> p j q w", j=2, q=4)
        for r in range(r0, r1):
            c = int(c_r[r]); off = int(off_r[r]); offl = off - col0
            prev, pstride = state["prev"], state["pstride"]
            u = rpool.tile([P, 8 * cmax], F32, tag="u", name=f"u{r}")[:, 0:8 * c]
            src = (prev[:, 0:2 * pstride].rearrange("p (j w) -> p j w", j=2)
                   [:, :, 0:c].unsqueeze(2).broadcast_to([P, 2, 4, c]))
            nc.vector.tensor_tensor(
                out=u.rearrange("p (j q w) -> p j q w", j=2, q=4),
                in0=src, in1=k4v[:, :, :, offl:offl + c], op=OP.mult)
            na = rpool.tile([P, 2 * cmax], F32, tag="na", name=f"na{r}")[:, 0:2 * c]
            nc.vector.tensor_add(na, u[:, 0:2 * c], u[:, 4 * c:6 * c])
            nc.vector.tensor_add(py3[:, 0:2, off:off + c],
                                 _r2(u[:, 2 * c:4 * c], c),
                                 _r2(u[:, 6 * c:8 * c], c))
            nc.vector.tensor_add(py3[:, 2, off:off + c],
                                 py3[:, 0, off:off + c], py3[:, 1, off:off + c])
            v_t = rpool.tile([P, 2 * cmax], F32, tag="v2",
                             name=f"v2_{r}")[:, 0:2 * c]
            nc.vector.tensor_scalar_max(v_t, na, 1e-20)
            state["prev"], state["pstride"] = v_t, c

    next_chunk = [0]
    st8_q = []

    def finish_tile(n, st8):
        pt = psum2.tile([P, 8], F32, tag="pt", name=f"pt{n}")
        nc.tensor.transpose(out=pt, in_=st8, identity=ident[0:8, 0:8])
        ci = int(chunk_of_col[4 * n])
        _, _, col0, w = chunks[ci]
        nc.vector.tensor_copy(
            out=_r2(ocat_ch[ci], w)[:, :, 4 * n - col0:4 * n - col0 + 4],
            in_=pt.rearrange("p (s c) -> p s c", s=2))
        while (next_chunk[0] < len(chunks)
               and chunks[next_chunk[0]][2] + chunks[next_chunk[0]][3]
               <= 4 * (n + 1)):
            phase_b_and_rounds(next_chunk[0])
            next_chunk[0] += 1

    for n in range(NTILE):
        if n == 2:
            emit_plane_loads()
        if n == 0:
            xt0 = [xt_pool.tile([P, 512], F32R, tag=f"xt0_{k}",
                                name=f"xt0_{k}") for k in range(4)]
            for k in range(4):
                nc.sync.dma_start(out=xt0[k], in_=xTv[:, k, 0:512])
            xtk = lambda k: xt0[k]
        else:
            xt = xt_pool.tile([P, 4, 512], F32R, tag="xt", name=f"xt{n}")
            nc.sync.dma_start(out=xt, in_=xTv[:, :, n * 512:(n + 1) * 512])
            xtk = lambda k: xt[:, k, :]
        ht = ht_pool.tile([P, 4, 512], F32R, tag="ht", name=f"ht{n}")
        for m in range(4):
            ph = psum.tile([P, 512], F32, tag=f"h{m}", name=f"h{m}_{n}")
            for k in range(4):
                nc.tensor.matmul(
                    ph,
                    lhsT=w1sb[k][:, m * 128:(m + 1) * 128],
                    rhs=xtk(k),
                    start=(k == 0), stop=(k == 3))
            nc.scalar.activation(out=ht[:, m, :], in_=ph, func=AF.Tanh,
                                 bias=b1sb[:, m:m + 1], scale=1.0)
        po = psum2.tile([2, 512], F32, tag="po", name=f"po{n}")
        for k in range(4):
            nc.tensor.matmul(po, lhsT=w2sb[:, 2 * k:2 * k + 2],
                             rhs=ht[:, k, :], start=(k == 0), stop=(k == 3))
        ots = sm_pool.tile([2, 512], F32, tag="ots", name=f"ots{n}")
        nc.vector.tensor_scalar(out=ots, in0=po, scalar1=b2sb, scalar2=None,
                                op0=OP.add)
        st8 = sm_pool.tile([8, 128], F32, tag="st8", name=f"st8{n}")
        nc.sync.dma_start(out=st8,
                          in_=ots.rearrange("s (c x) -> s c x", c=4))
        st8_q.append((n, st8))
        if len(st8_q) >= 2:
            finish_tile(*st8_q.pop(0))

    while st8_q:
        finish_tile(*st8_q.pop(0))
    while next_chunk[0] < len(chunks):
        phase_b_and_rounds(next_chunk[0])
        next_chunk[0] += 1

    # one Ln pass over [py0|py1|s], then out = ln(py) - ln(s)
    lnp = singles.tile([P, 3 * Qc], F32, tag="lnp")
    nc.scalar.activation(out=lnp, in_=pyt, func=AF.Ln)
    lnp3 = lnp.rearrange("p (s w) -> p s w", s=3)
    nc.vector.tensor_tensor(out=out3, in0=lnp3[:, 0:2, :],
                            in1=lnp3[:, 2:3, :].broadcast_to([P, 2, Qc]),
                            op=OP.subtract)
    nc.sync.dma_start(out=dram["out"], in_=outt)


def _build_nc(lay, repeat=1):
    from contextlib import ExitStack
    nc = bacc.Bacc("TRN2", target_bir_lowering=False, debug=False,
                   num_devices=NCORES)
    Qc, Q = lay["Qc"], lay["Q"]
    dram = {}
    def din(name, shape, dt=F32):
        dram[name] = nc.dram_tensor(name, shape, dt, kind="ExternalInput").ap()
    din("xT", [NF, Q], F32R)
    din("w1", [NF, NH], F32R)
    din("b1r", [P, 4])
    din("w2r", [P, 8], F32R)
    din("b2", [2])
    din("og0", [P, 2 * Qc])
    din("og1", [P, 2 * Qc])
    din("tg0", [P, 2 * Qc])
    din("tg1", [P, 2 * Qc])
    din("sgn", [P, Qc])
    din("ig", [P, 64])
    dram["out"] = nc.dram_tensor("out", [P, 2 * Qc], F32,
                                 kind="ExternalOutput").ap()
    with tile.TileContext(nc) as tc:
        with ExitStack() as ctx:
            _kernel_body(ctx, tc, lay, dram, repeat=repeat)
    nc.compile()
    return nc


_NC_CACHE = {}


def _get_nc(lay):
    key = tuple(int(x) for x in lay["c_r"])
    if key not in _NC_CACHE:
        _NC_CACHE[key] = _build_nc(lay)
    return _NC_CACHE[key]


# ---------------------------------------------------------------------------
# entry point
# ---------------------------------------------------------------------------

def kernel(corr, kc, FM, W1, b1, W2, b2, trans_logits, obs_logits, init_logits,
           _want_results_only=True, _trace=False):
    inputs = dict(corr=corr, kc=kc, FM=FM, W1=W1, b1=b1, W2=W2, b2=b2,
                  trans_logits=trans_logits, obs_logits=obs_logits,
                  init_logits=init_logits)
    lay = _build_layout(kc)
    nc = _get_nc(lay)
    per_core, shared = _build_host_tensors(inputs, lay)

    in_maps = []
    for m in range(NCORES):
        c = per_core[m]
        in_maps.append(dict(
            xT=c["xT"], w1=shared["w1"], b1r=shared["b1r"], w2r=shared["w2r"],
            b2=shared["b2"], og0=c["og0"], og1=c["og1"], tg0=c["tg0"],
            tg1=c["tg1"], sgn=c["sgn"], ig=c["ig"]))

    res = run_bass_kernel_spmd(nc, in_maps, core_ids=list(range(NCORES)),
                               trace=_trace)

    Qc, Q = lay["Qc"], lay["Q"]
    out = np.zeros((B * T, 2), dtype=np.float32)
    J = np.arange(Q) // 128
    p = np.arange(Q) % 128
    for m in range(NCORES):
        OUT = res.results[m]["out"]
        g = per_core[m]["perm"]; v = per_core[m]["valid"]
        out[g[v], 0] = OUT[p[v], J[v]]
        out[g[v], 1] = OUT[p[v], Qc + J[v]]
    out = out.reshape(B, T, 2)
    if _want_results_only:
        return out
    return out, res
